# revision 14
# baseline (speedup 1.0000x reference)
"""DeepseekV3 MoE gate (moe_routing) for 8x TRN2 NeuronCores.

Sharding: data-parallel over tokens. Each core gets a 2048-token shard of x
(pre-transposed on host to [H, Tc] so both matmul operands DMA naturally with
the contraction dim on partitions); the small gate weight [7168, 256] and bias
are replicated.

Per-core pipeline, per 128-token tile:
  PE:   logits = xT_chunk.T @ W_chunk accumulated over 56 K-chunks in PSUM
  ACT:  s0 = sigmoid(logits)  (PSUM -> SBUF eviction fused)
  DVE:  b = s0 + bias; per-group Max8 -> top-2 sums -> group top-4 threshold
        -> additive mask; Max8/MaxIndex8 over masked scores -> top-8 experts
  Q7:   rank-payload local_scatter; z = payload + s0; Max8(z) recovers the
        selected s0 in rank order (exact ordering, no gather needed)
  DVE:  normalize * 2.5, emit weights + indices
"""

import sys

if "/opt/trn_rl_repo" not in sys.path:
    sys.path.insert(0, "/opt/trn_rl_repo")

from contextlib import ExitStack

import numpy as np

import concourse.bass as bass
import concourse.mybir as mybir
import concourse.tile as tile
from concourse import bacc
from concourse._compat import with_exitstack

H = 7168
E = 256
G = 8
EPG = E // G  # 32
K = 8
NEG = -1.0e30
ROUTE_SCALE = 2.5
P = 128

N_CORES = 8
T_FULL = 16384
T_CORE = T_FULL // N_CORES  # 2048

MODE = "f32"  # "f32r" | "f32" | "f16x3"
BLK = 512


def np_algo_reference(x, W, bias):
    """Numpy mirror of the kernel algorithm (for validation in tests)."""
    x = x.astype(np.float32)
    T = x.shape[0]
    logits = (x.astype(np.float64) @ W.astype(np.float64)).astype(np.float32)
    s0 = (1.0 / (1.0 + np.exp(-logits.astype(np.float64)))).astype(np.float32)
    b = s0 + bias.astype(np.float32)
    bg = b.reshape(T, G, EPG)
    top2 = np.sort(bg, axis=-1)[:, :, -2:]
    gs = (top2[:, :, 0] + top2[:, :, 1]).astype(np.float32)
    gsort = np.sort(gs, axis=-1)[:, ::-1]
    thresh = gsort[:, 3:4]
    pen = np.where(gs >= thresh, np.float32(0.0), np.float32(NEG))
    ms = b + np.repeat(pen, EPG, axis=1)
    order = np.argsort(-ms, axis=-1, kind="stable")[:, :K]
    s0sel = np.take_along_axis(s0, order, axis=-1)
    q = np.arange(K, 0, -1).astype(np.float32)
    z = (q[None, :] + s0sel).astype(np.float32)
    s0sel_rt = (z - q[None, :]).astype(np.float32)
    ssum = s0sel_rt.sum(-1, keepdims=True, dtype=np.float32)
    wts = (s0sel_rt * ((np.float32(1.0) / ssum) * np.float32(ROUTE_SCALE))).astype(
        np.float32
    )
    return wts, order.astype(np.int32)


@with_exitstack
def _gate_kernel(
    ctx: ExitStack,
    tc: tile.TileContext,
    outs,
    ins,
    T_core: int,
    BLK: int = 512,
    mode: str = "f32r",
    repeat: int = 1,
):
    nc = tc.nc
    wts_d, sel_d = outs
    if mode == "f16x3":
        xh_d, xl_d, wh_d, wl_d, bias_d = ins
    else:
        xT_d, w_d, bias_d = ins

    n_blk = T_core // BLK
    n_sub = BLK // P
    n_k = H // P  # 56
    KPACK = 4  # k-chunks per x DMA (fewer, larger DMAs)
    assert n_k % KPACK == 0
    n_tiles = T_core // P

    f32 = mybir.dt.float32
    f16 = mybir.dt.float16
    assert T_core % BLK == 0 and BLK % P == 0

    const = ctx.enter_context(tc.tile_pool(name="const", bufs=1))
    wpool = ctx.enter_context(tc.tile_pool(name="wpool", bufs=1))
    xpool = ctx.enter_context(tc.tile_pool(name="xpool", bufs=4))
    ppool = ctx.enter_context(tc.tile_pool(name="ppool", bufs=8, space="PSUM"))
    spool = ctx.enter_context(tc.tile_pool(name="spool", bufs=3))
    opool = ctx.enter_context(tc.tile_pool(name="opool", bufs=3))

    # ---- constants ----
    bias_bc = const.tile([P, E], f32)
    nc.sync.dma_start(bias_bc[:], bias_d.unsqueeze(0).to_broadcast([P, E]))

    qrow32 = const.tile([P, K], f32)
    for k in range(K):
        nc.vector.memset(qrow32[:, k : k + 1], float(K - k))

    # ---- resident weights ----
    if mode == "f16x3":
        w_all_h = wpool.tile([P, n_k, E], f16)
        w_all_l = wpool.tile([P, n_k, E], f16)
        nc.sync.dma_start(w_all_h[:], wh_d.rearrange("(k p) e -> p k e", p=P))
        nc.sync.dma_start(w_all_l[:], wl_d.rearrange("(k p) e -> p k e", p=P))
    else:
        mdt = mybir.dt.float32r if mode == "f32r" else f32
        w_all = wpool.tile([P, n_k, E], mdt)
        w_view = w_d.rearrange("(k p) e -> p k e", p=P)
        wsplit = 14  # k-chunks per W DMA: let early chunks land first
        for k0 in range(0, n_k, wsplit):
            k1 = min(k0 + wsplit, n_k)
            nc.sync.dma_start(w_all[:, k0:k1, :], w_view[:, k0:k1, :])

    # ---- main loop ----
    for rep, tb in [(r, b) for r in range(repeat) for b in range(n_blk)]:
        t0 = tb * BLK
        psums = []
        for s in range(n_sub):
            pt = ppool.tile([P, E], f32, name=f"psum_{rep}_{tb}_{s}", tag="psum")
            psums.append(pt)

        for k in range(n_k):
            start = k == 0
            stop = k == n_k - 1
            if mode == "f16x3":
                xch = xpool.tile([P, BLK], f16, tag="xch")
                xcl = xpool.tile([P, BLK], f16, tag="xcl")
                nc.sync.dma_start(xch[:], xh_d[k * P : (k + 1) * P, t0 : t0 + BLK])
                nc.sync.dma_start(xcl[:], xl_d[k * P : (k + 1) * P, t0 : t0 + BLK])
                for s in range(n_sub):
                    lh = xch[:, s * P : (s + 1) * P]
                    ll = xcl[:, s * P : (s + 1) * P]
                    nc.tensor.matmul(
                        psums[s][:], lh, w_all_h[:, k, :], start=start, stop=False
                    )
                    nc.tensor.matmul(
                        psums[s][:], ll, w_all_h[:, k, :], start=False, stop=False
                    )
                    nc.tensor.matmul(
                        psums[s][:], lh, w_all_l[:, k, :], start=False, stop=stop
                    )
            else:
                xc = xpool.tile([P, BLK], mdt, tag="xch")
                nc.sync.dma_start(xc[:], xT_d[k * P : (k + 1) * P, t0 : t0 + BLK])
                for s in range(n_sub):
                    nc.tensor.matmul(
                        psums[s][:],
                        xc[:, s * P : (s + 1) * P],
                        w_all[:, k, :],
                        start=start,
                        stop=stop,
                    )

        for s in range(n_sub):
            trow = t0 + s * P
            s0 = spool.tile([P, E], f32, tag="s0")
            nc.scalar.activation(
                s0[:], psums[s][:], mybir.ActivationFunctionType.Sigmoid
            )
            b = spool.tile([P, E], f32, tag="b")
            nc.vector.tensor_add(b[:], s0[:], bias_bc[:])
            gmax = opool.tile([P, G * 8], f32, tag="gmax")
            for g in range(G):
                nc.vector.max(
                    out=gmax[:, g * 8 : (g + 1) * 8],
                    in_=b[:, g * EPG : (g + 1) * EPG],
                )
            gv = gmax[:].rearrange("p (g c) -> p g c", g=G)
            gs = opool.tile([P, G], f32, tag="gs")
            nc.vector.tensor_add(gs[:], gv[:, :, 0], gv[:, :, 1])
            gtop = opool.tile([P, 8], f32, tag="gtop")
            nc.vector.max(out=gtop[:], in_=gs[:])
            pen = opool.tile([P, G], f32, tag="pen")
            nc.vector.tensor_scalar(
                pen[:],
                gs[:],
                gtop[:, 3:4],
                None,
                op0=mybir.AluOpType.is_ge,
            )
            nc.vector.tensor_scalar(
                pen[:],
                pen[:],
                1.0,
                -NEG,
                op0=mybir.AluOpType.subtract,
                op1=mybir.AluOpType.mult,
            )
            ms = spool.tile([P, E], f32, tag="ms")
            pen_bc = pen[:].unsqueeze(2).to_broadcast([P, G, EPG])
            nc.vector.tensor_add(
                ms[:].rearrange("p (g c) -> p g c", g=G),
                b[:].rearrange("p (g c) -> p g c", g=G),
                pen_bc,
            )
            vals8 = opool.tile([P, K], f32, tag="vals8")
            nc.vector.max(out=vals8[:], in_=ms[:])
            idxu = opool.tile([P, K], mybir.dt.uint16, tag="idxu")
            nc.vector.max_index(idxu[:], vals8[:], ms[:])
            # z[p,e] = s0[p,e] + #{k : ms[p,e] >= vals8[p,k]}
            # selected rank-r expert lands in band (8-r, 9-r); others in (0,1)
            z = spool.tile([P, E], f32, tag="z")
            nc.vector.scalar_tensor_tensor(
                z[:],
                ms[:],
                vals8[:, 0:1],
                s0[:],
                op0=mybir.AluOpType.is_ge,
                op1=mybir.AluOpType.add,
            )
            for k in range(1, K):
                nc.vector.scalar_tensor_tensor(
                    z[:],
                    ms[:],
                    vals8[:, k : k + 1],
                    z[:],
                    op0=mybir.AluOpType.is_ge,
                    op1=mybir.AluOpType.add,
                )
            zv = opool.tile([P, K], f32, tag="zv")
            nc.vector.max(out=zv[:], in_=z[:])
            s0sel = opool.tile([P, K], f32, tag="s0sel")
            nc.vector.tensor_sub(s0sel[:], zv[:], qrow32[:])
            ssum = opool.tile([P, 1], f32, tag="ssum")
            nc.vector.tensor_reduce(
                ssum[:], s0sel[:], axis=mybir.AxisListType.X, op=mybir.AluOpType.add
            )
            rec = opool.tile([P, 1], f32, tag="rec")
            nc.vector.reciprocal(rec[:], ssum[:])
            wts = opool.tile([P, K], f32, tag="wts")
            nc.vector.tensor_scalar(
                wts[:],
                s0sel[:],
                rec[:],
                ROUTE_SCALE,
                op0=mybir.AluOpType.mult,
                op1=mybir.AluOpType.mult,
            )
            seli = opool.tile([P, K], mybir.dt.int32, tag="seli")
            nc.vector.tensor_copy(seli[:], idxu[:])
            nc.sync.dma_start(wts_d[trow : trow + P, :], wts[:])
            nc.sync.dma_start(sel_d[trow : trow + P, :], seli[:])


_NC_CACHE = {}


def _build(mode=MODE, t_core=T_CORE, blk=BLK, repeat=1):
    key = (mode, t_core, blk, repeat)
    if key in _NC_CACHE:
        return _NC_CACHE[key]
    nc = bacc.Bacc("TRN2", target_bir_lowering=False, debug=False)
    f32 = mybir.dt.float32
    f16 = mybir.dt.float16
    if mode == "f16x3":
        ins = [
            nc.dram_tensor("xh", [H, t_core], f16, kind="ExternalInput").ap(),
            nc.dram_tensor("xl", [H, t_core], f16, kind="ExternalInput").ap(),
            nc.dram_tensor("wh", [H, E], f16, kind="ExternalInput").ap(),
            nc.dram_tensor("wl", [H, E], f16, kind="ExternalInput").ap(),
            nc.dram_tensor("bias", [E], f32, kind="ExternalInput").ap(),
        ]
    else:
        mdt = mybir.dt.float32r if mode == "f32r" else f32
        ins = [
            nc.dram_tensor("xT", [H, t_core], mdt, kind="ExternalInput").ap(),
            nc.dram_tensor("w", [H, E], mdt, kind="ExternalInput").ap(),
            nc.dram_tensor("bias", [E], f32, kind="ExternalInput").ap(),
        ]
    outs = [
        nc.dram_tensor("wts", [t_core, K], f32, kind="ExternalOutput").ap(),
        nc.dram_tensor("sel", [t_core, K], mybir.dt.int32, kind="ExternalOutput").ap(),
    ]
    with tile.TileContext(nc) as tc:
        _gate_kernel(tc, outs, ins, T_core=t_core, BLK=blk, mode=mode, repeat=repeat)
    nc.compile()
    _NC_CACHE[key] = nc
    return nc


def _make_in_maps(x, W_gate, bias, mode=MODE):
    x = np.asarray(x, dtype=np.float32)
    W_gate = np.asarray(W_gate, dtype=np.float32)
    bias = np.asarray(bias, dtype=np.float32)
    in_maps = []
    if mode == "f16x3":
        Wh = W_gate.astype(np.float16)
        Wl = (W_gate - Wh.astype(np.float32)).astype(np.float16)
        for c in range(N_CORES):
            xT = x[c * T_CORE : (c + 1) * T_CORE].T
            xh = np.ascontiguousarray(xT.astype(np.float16))
            xl = np.ascontiguousarray(
                (xT - xh.astype(np.float32)).astype(np.float16)
            )
            in_maps.append({"xh": xh, "xl": xl, "wh": Wh, "wl": Wl, "bias": bias})
    else:
        for c in range(N_CORES):
            xT = np.ascontiguousarray(x[c * T_CORE : (c + 1) * T_CORE].T)
            in_maps.append({"xT": xT, "w": W_gate, "bias": bias})
    return in_maps


_NEFF_CACHE_DIR = "/tmp/bass_neff_cache"
_neff_cache_installed = False


def _install_neff_cache():
    """Cache compiled NEFFs by BIR hash so repeat runs skip walrus."""
    global _neff_cache_installed
    if _neff_cache_installed:
        return
    import hashlib
    import os
    import shutil

    from concourse import bass2jax, bass_utils

    orig = bass_utils.compile_bir_kernel

    def cached(bir_json, tmpdir, neff_name="file.neff"):
        h = hashlib.sha256(bir_json).hexdigest()[:24]
        os.makedirs(_NEFF_CACHE_DIR, exist_ok=True)
        cpath = os.path.join(_NEFF_CACHE_DIR, h + ".neff")
        out = os.path.join(tmpdir, neff_name)
        if os.path.exists(cpath):
            shutil.copy(cpath, out)
            return out
        p = orig(bir_json, tmpdir, neff_name)
        try:
            shutil.copy(p, cpath)
        except OSError:
            pass
        return p

    bass2jax.compile_bir_kernel = cached
    _neff_cache_installed = True


def run_on_hw(x, W_gate, bias, mode=MODE, trace=False, **kwargs):
    from concourse import bass_utils

    _install_neff_cache()
    nc = _build(mode)
    in_maps = _make_in_maps(x, W_gate, bias, mode)
    res = bass_utils.run_bass_kernel_spmd(
        nc, in_maps, list(range(N_CORES)), trace=trace, **kwargs
    )
    wts = np.concatenate([r["wts"] for r in res.results], axis=0)
    sel = np.concatenate([r["sel"] for r in res.results], axis=0)
    return (wts.astype(np.float32), sel.astype(np.int32)), res


def kernel(x, W_gate, bias):
    (wts, sel), _ = run_on_hw(x, W_gate, bias, MODE)
    return wts, sel


# revision 16
# speedup vs baseline: 436.0863x; 436.0863x over previous
"""DeepseekV3 MoE gate (moe_routing) for 8x TRN2 NeuronCores.

Sharding: data-parallel over tokens. Each core gets a 2048-token shard of x
(pre-transposed on host to [H, Tc] so both matmul operands DMA naturally with
the contraction dim on partitions); the small gate weight [7168, 256] and bias
are replicated.

Per-core pipeline, per 128-token tile:
  PE:   logits = xT_chunk.T @ W_chunk accumulated over 56 K-chunks in PSUM
  ACT:  s0 = sigmoid(logits)  (PSUM -> SBUF eviction fused)
  DVE:  b = s0 + bias; per-group Max8 -> top-2 sums -> group top-4 threshold
        -> additive mask; Max8/MaxIndex8 over masked scores -> top-8 experts
  Q7:   rank-payload local_scatter; z = payload + s0; Max8(z) recovers the
        selected s0 in rank order (exact ordering, no gather needed)
  DVE:  normalize * 2.5, emit weights + indices
"""

import sys

if "/opt/trn_rl_repo" not in sys.path:
    sys.path.insert(0, "/opt/trn_rl_repo")

from contextlib import ExitStack

import numpy as np

import concourse.bass as bass
import concourse.mybir as mybir
import concourse.tile as tile
from concourse import bacc
from concourse._compat import with_exitstack

H = 7168
E = 256
G = 8
EPG = E // G  # 32
K = 8
NEG = -1.0e30
ROUTE_SCALE = 2.5
P = 128

N_CORES = 8
T_FULL = 16384
T_CORE = T_FULL // N_CORES  # 2048

MODE = "f16x3"  # "f32r" | "f32" | "f16x3"
BLK = 512


def np_algo_reference(x, W, bias):
    """Numpy mirror of the kernel algorithm (for validation in tests)."""
    x = x.astype(np.float32)
    T = x.shape[0]
    logits = (x.astype(np.float64) @ W.astype(np.float64)).astype(np.float32)
    s0 = (1.0 / (1.0 + np.exp(-logits.astype(np.float64)))).astype(np.float32)
    b = s0 + bias.astype(np.float32)
    bg = b.reshape(T, G, EPG)
    top2 = np.sort(bg, axis=-1)[:, :, -2:]
    gs = (top2[:, :, 0] + top2[:, :, 1]).astype(np.float32)
    gsort = np.sort(gs, axis=-1)[:, ::-1]
    thresh = gsort[:, 3:4]
    pen = np.where(gs >= thresh, np.float32(0.0), np.float32(NEG))
    ms = b + np.repeat(pen, EPG, axis=1)
    order = np.argsort(-ms, axis=-1, kind="stable")[:, :K]
    s0sel = np.take_along_axis(s0, order, axis=-1)
    q = np.arange(K, 0, -1).astype(np.float32)
    z = (q[None, :] + s0sel).astype(np.float32)
    s0sel_rt = (z - q[None, :]).astype(np.float32)
    ssum = s0sel_rt.sum(-1, keepdims=True, dtype=np.float32)
    wts = (s0sel_rt * ((np.float32(1.0) / ssum) * np.float32(ROUTE_SCALE))).astype(
        np.float32
    )
    return wts, order.astype(np.int32)


@with_exitstack
def _gate_kernel(
    ctx: ExitStack,
    tc: tile.TileContext,
    outs,
    ins,
    T_core: int,
    BLK: int = 512,
    mode: str = "f32r",
    repeat: int = 1,
):
    nc = tc.nc
    wts_d, sel_d = outs
    if mode == "f16x3":
        xh_d, xl_d, wh_d, wl_d, bias_d = ins
    else:
        xT_d, w_d, bias_d = ins

    n_blk = T_core // BLK
    n_sub = BLK // P
    n_k = H // P  # 56
    KPACK = 4  # k-chunks per x DMA (fewer, larger DMAs)
    assert n_k % KPACK == 0
    n_tiles = T_core // P

    f32 = mybir.dt.float32
    f16 = mybir.dt.float16
    assert T_core % BLK == 0 and BLK % P == 0

    const = ctx.enter_context(tc.tile_pool(name="const", bufs=1))
    wpool = ctx.enter_context(tc.tile_pool(name="wpool", bufs=1))
    xpool = ctx.enter_context(tc.tile_pool(name="xpool", bufs=4))
    ppool = ctx.enter_context(tc.tile_pool(name="ppool", bufs=8, space="PSUM"))
    spool = ctx.enter_context(tc.tile_pool(name="spool", bufs=3))
    opool = ctx.enter_context(tc.tile_pool(name="opool", bufs=3))

    # ---- constants ----
    bias_bc = const.tile([P, E], f32)
    nc.sync.dma_start(bias_bc[:], bias_d.unsqueeze(0).to_broadcast([P, E]))

    qrow32 = const.tile([P, K], f32)
    for k in range(K):
        nc.vector.memset(qrow32[:, k : k + 1], float(K - k))

    # ---- resident weights ----
    if mode == "f16x3":
        w_all_h = wpool.tile([P, n_k, E], f16)
        w_all_l = wpool.tile([P, n_k, E], f16)
        nc.sync.dma_start(w_all_h[:], wh_d.rearrange("(k p) e -> p k e", p=P))
        nc.sync.dma_start(w_all_l[:], wl_d.rearrange("(k p) e -> p k e", p=P))
    else:
        mdt = mybir.dt.float32r if mode == "f32r" else f32
        w_all = wpool.tile([P, n_k, E], mdt)
        w_view = w_d.rearrange("(k p) e -> p k e", p=P)
        wsplit = 14  # k-chunks per W DMA: let early chunks land first
        for k0 in range(0, n_k, wsplit):
            k1 = min(k0 + wsplit, n_k)
            nc.sync.dma_start(w_all[:, k0:k1, :], w_view[:, k0:k1, :])

    # ---- main loop ----
    for rep, tb in [(r, b) for r in range(repeat) for b in range(n_blk)]:
        t0 = tb * BLK
        psums = []
        for s in range(n_sub):
            pt = ppool.tile([P, E], f32, name=f"psum_{rep}_{tb}_{s}", tag="psum")
            psums.append(pt)

        for k in range(n_k):
            start = k == 0
            stop = k == n_k - 1
            if mode == "f16x3":
                xch = xpool.tile([P, BLK], f16, tag="xch")
                xcl = xpool.tile([P, BLK], f16, tag="xcl")
                nc.sync.dma_start(xch[:], xh_d[k * P : (k + 1) * P, t0 : t0 + BLK])
                nc.sync.dma_start(xcl[:], xl_d[k * P : (k + 1) * P, t0 : t0 + BLK])
                for s in range(n_sub):
                    lh = xch[:, s * P : (s + 1) * P]
                    ll = xcl[:, s * P : (s + 1) * P]
                    # xh stationary twice in a row -> cheaper weight reload
                    nc.tensor.matmul(
                        psums[s][:], lh, w_all_h[:, k, :], start=start, stop=False
                    )
                    nc.tensor.matmul(
                        psums[s][:], lh, w_all_l[:, k, :], start=False, stop=False
                    )
                    nc.tensor.matmul(
                        psums[s][:], ll, w_all_h[:, k, :], start=False, stop=stop
                    )
            else:
                xc = xpool.tile([P, BLK], mdt, tag="xch")
                nc.sync.dma_start(xc[:], xT_d[k * P : (k + 1) * P, t0 : t0 + BLK])
                for s in range(n_sub):
                    nc.tensor.matmul(
                        psums[s][:],
                        xc[:, s * P : (s + 1) * P],
                        w_all[:, k, :],
                        start=start,
                        stop=stop,
                    )

        for s in range(n_sub):
            trow = t0 + s * P
            s0 = spool.tile([P, E], f32, tag="s0")
            nc.scalar.activation(
                s0[:], psums[s][:], mybir.ActivationFunctionType.Sigmoid
            )
            b = spool.tile([P, E], f32, tag="b")
            nc.vector.tensor_add(b[:], s0[:], bias_bc[:])
            gmax = opool.tile([P, G * 8], f32, tag="gmax")
            for g in range(G):
                nc.vector.max(
                    out=gmax[:, g * 8 : (g + 1) * 8],
                    in_=b[:, g * EPG : (g + 1) * EPG],
                )
            gv = gmax[:].rearrange("p (g c) -> p g c", g=G)
            gs = opool.tile([P, G], f32, tag="gs")
            nc.vector.tensor_add(gs[:], gv[:, :, 0], gv[:, :, 1])
            gtop = opool.tile([P, 8], f32, tag="gtop")
            nc.vector.max(out=gtop[:], in_=gs[:])
            pen = opool.tile([P, G], f32, tag="pen")
            nc.vector.tensor_scalar(
                pen[:],
                gs[:],
                gtop[:, 3:4],
                None,
                op0=mybir.AluOpType.is_ge,
            )
            nc.vector.tensor_scalar(
                pen[:],
                pen[:],
                1.0,
                -NEG,
                op0=mybir.AluOpType.subtract,
                op1=mybir.AluOpType.mult,
            )
            ms = spool.tile([P, E], f32, tag="ms")
            pen_bc = pen[:].unsqueeze(2).to_broadcast([P, G, EPG])
            nc.vector.tensor_add(
                ms[:].rearrange("p (g c) -> p g c", g=G),
                b[:].rearrange("p (g c) -> p g c", g=G),
                pen_bc,
            )
            vals8 = opool.tile([P, K], f32, tag="vals8")
            nc.vector.max(out=vals8[:], in_=ms[:])
            idxu = opool.tile([P, K], mybir.dt.uint16, tag="idxu")
            nc.vector.max_index(idxu[:], vals8[:], ms[:])
            # z[p,e] = s0[p,e] + #{k : ms[p,e] >= vals8[p,k]}
            # selected rank-r expert lands in band (8-r, 9-r); others in (0,1)
            z = spool.tile([P, E], f32, tag="z")
            nc.vector.scalar_tensor_tensor(
                z[:],
                ms[:],
                vals8[:, 0:1],
                s0[:],
                op0=mybir.AluOpType.is_ge,
                op1=mybir.AluOpType.add,
            )
            for k in range(1, K):
                nc.vector.scalar_tensor_tensor(
                    z[:],
                    ms[:],
                    vals8[:, k : k + 1],
                    z[:],
                    op0=mybir.AluOpType.is_ge,
                    op1=mybir.AluOpType.add,
                )
            zv = opool.tile([P, K], f32, tag="zv")
            nc.vector.max(out=zv[:], in_=z[:])
            s0sel = opool.tile([P, K], f32, tag="s0sel")
            nc.vector.tensor_sub(s0sel[:], zv[:], qrow32[:])
            ssum = opool.tile([P, 1], f32, tag="ssum")
            nc.vector.tensor_reduce(
                ssum[:], s0sel[:], axis=mybir.AxisListType.X, op=mybir.AluOpType.add
            )
            rec = opool.tile([P, 1], f32, tag="rec")
            nc.vector.reciprocal(rec[:], ssum[:])
            wts = opool.tile([P, K], f32, tag="wts")
            nc.vector.tensor_scalar(
                wts[:],
                s0sel[:],
                rec[:],
                ROUTE_SCALE,
                op0=mybir.AluOpType.mult,
                op1=mybir.AluOpType.mult,
            )
            seli = opool.tile([P, K], mybir.dt.int32, tag="seli")
            nc.vector.tensor_copy(seli[:], idxu[:])
            nc.sync.dma_start(wts_d[trow : trow + P, :], wts[:])
            nc.sync.dma_start(sel_d[trow : trow + P, :], seli[:])


_NC_CACHE = {}


def _build(mode=MODE, t_core=T_CORE, blk=BLK, repeat=1):
    key = (mode, t_core, blk, repeat)
    if key in _NC_CACHE:
        return _NC_CACHE[key]
    nc = bacc.Bacc("TRN2", target_bir_lowering=False, debug=False)
    f32 = mybir.dt.float32
    f16 = mybir.dt.float16
    if mode == "f16x3":
        ins = [
            nc.dram_tensor("xh", [H, t_core], f16, kind="ExternalInput").ap(),
            nc.dram_tensor("xl", [H, t_core], f16, kind="ExternalInput").ap(),
            nc.dram_tensor("wh", [H, E], f16, kind="ExternalInput").ap(),
            nc.dram_tensor("wl", [H, E], f16, kind="ExternalInput").ap(),
            nc.dram_tensor("bias", [E], f32, kind="ExternalInput").ap(),
        ]
    else:
        mdt = mybir.dt.float32r if mode == "f32r" else f32
        ins = [
            nc.dram_tensor("xT", [H, t_core], mdt, kind="ExternalInput").ap(),
            nc.dram_tensor("w", [H, E], mdt, kind="ExternalInput").ap(),
            nc.dram_tensor("bias", [E], f32, kind="ExternalInput").ap(),
        ]
    outs = [
        nc.dram_tensor("wts", [t_core, K], f32, kind="ExternalOutput").ap(),
        nc.dram_tensor("sel", [t_core, K], mybir.dt.int32, kind="ExternalOutput").ap(),
    ]
    with tile.TileContext(nc) as tc:
        _gate_kernel(tc, outs, ins, T_core=t_core, BLK=blk, mode=mode, repeat=repeat)
    nc.compile()
    _NC_CACHE[key] = nc
    return nc


def _make_in_maps(x, W_gate, bias, mode=MODE):
    x = np.asarray(x, dtype=np.float32)
    W_gate = np.asarray(W_gate, dtype=np.float32)
    bias = np.asarray(bias, dtype=np.float32)
    in_maps = []
    if mode == "f16x3":
        Wh = W_gate.astype(np.float16)
        Wl = (W_gate - Wh.astype(np.float32)).astype(np.float16)
        for c in range(N_CORES):
            xT = x[c * T_CORE : (c + 1) * T_CORE].T
            xh = np.ascontiguousarray(xT.astype(np.float16))
            xl = np.ascontiguousarray(
                (xT - xh.astype(np.float32)).astype(np.float16)
            )
            in_maps.append({"xh": xh, "xl": xl, "wh": Wh, "wl": Wl, "bias": bias})
    else:
        for c in range(N_CORES):
            xT = np.ascontiguousarray(x[c * T_CORE : (c + 1) * T_CORE].T)
            in_maps.append({"xT": xT, "w": W_gate, "bias": bias})
    return in_maps


_NEFF_CACHE_DIR = "/tmp/bass_neff_cache"
_neff_cache_installed = False


def _install_neff_cache():
    """Cache compiled NEFFs by BIR hash so repeat runs skip walrus."""
    global _neff_cache_installed
    if _neff_cache_installed:
        return
    import hashlib
    import os
    import shutil

    from concourse import bass2jax, bass_utils

    orig = bass_utils.compile_bir_kernel

    def cached(bir_json, tmpdir, neff_name="file.neff"):
        h = hashlib.sha256(bir_json).hexdigest()[:24]
        os.makedirs(_NEFF_CACHE_DIR, exist_ok=True)
        cpath = os.path.join(_NEFF_CACHE_DIR, h + ".neff")
        out = os.path.join(tmpdir, neff_name)
        if os.path.exists(cpath):
            shutil.copy(cpath, out)
            return out
        p = orig(bir_json, tmpdir, neff_name)
        try:
            shutil.copy(p, cpath)
        except OSError:
            pass
        return p

    bass2jax.compile_bir_kernel = cached
    _neff_cache_installed = True


def run_on_hw(x, W_gate, bias, mode=MODE, trace=False, **kwargs):
    from concourse import bass_utils

    _install_neff_cache()
    nc = _build(mode)
    in_maps = _make_in_maps(x, W_gate, bias, mode)
    res = bass_utils.run_bass_kernel_spmd(
        nc, in_maps, list(range(N_CORES)), trace=trace, **kwargs
    )
    wts = np.concatenate([r["wts"] for r in res.results], axis=0)
    sel = np.concatenate([r["sel"] for r in res.results], axis=0)
    return (wts.astype(np.float32), sel.astype(np.int32)), res


def kernel(x, W_gate, bias):
    (wts, sel), _ = run_on_hw(x, W_gate, bias, MODE)
    return wts, sel
